# revision 40
# baseline (speedup 1.0000x reference)
"""Single-head attention (B=8, S=4096, D=1024, DK=DV=128) on 8 TRN2 NeuronCores.

Sharding: data-parallel over batch - one batch element per core, the three
Linear weights replicated. No collectives.

Measured (core 0 traced, 8 cores running): ~185-190us at the chip's
nominal clock (the chip's P0 power state adds up to ~20% run-to-run),
rel_err 1.10e-2 vs the f32 reference (tolerance 2e-2); v2 baseline was
212-221us at rel_err 1.7e-3.

Trace-driven changes from the v2 baseline:

1. ScalarE is the steady-state bottleneck: each [128,1024] exp ACTIVATE
   costs (N+352)/1.2 ns and the per-call overhead never pipelines away,
   so 128 ACTs = ~129us effective minimum. Everything else is scheduled
   around keeping that ACT stream dense:
   - all x-chunk load DMA triggers moved off the scalar queue onto the
     sync HWDGE ring only (one ring's DMA already fans out to all 16 SDMA
     engines; sequential c-splits keep projection subtile deps early);
     weights/biases on gpsimd SWDGE.
   - deferred-PV pipeline: each group's exp output (pt) is stashed in a
     deep SBUF pool (34 tiles) and its P^T@[V|1] matmuls are emitted LAG=3
     groups later (2*LAG at q-block boundaries, covering the attn_end DVE
     chain), so the PE-FIFO head never waits on a fresh ACT or on DVE and
     the ACT chain depends only on scores matmuls. Steady state measures
     ~1000ns/ACT (the floor).
   - qb0's scores+exp ride the DMA-bound load front SOLO, so attn_end(0)
     fires right at front-end and qb1's deferred PV (ops-WAR-gated on it)
     overlaps qb1's own gapless exp sweep instead of piling up behind a
     late attn_end (interleaving qb0+qb1 in the front was measured worse:
     it pushes attn_end(0) ~16us later and stalls ScalarE ~6us).

2. FP8_QK: query/key activations AND Wq/Wk are cast host-side to fp8-e4m3
   (TRN float8e4; values well under the 240 max). The Q/K projections run
   as fp8 DoubleRow matmuls (contraction pairs of d-chunks, 2 MACs/cycle):
   4 pair-MMs per 512-block instead of 8, halving Q/K projection PE time,
   and the Q/K input DMA bytes halve (the front is HBM-bound at ~300-410
   GB/s/core, so 16MB instead of 24MB directly shortens it). V path stays
   bf16: the output is a near-uniform softmax average, so V-side
   quantization error does not average out (numpy-sim: V-fp8 1.9e-2 vs
   QK-fp8 1.1e-2, both calibrated within 1% of HW).

3. Host staging is chunk-contiguous: x^T is pre-arranged per load-chunk as
   [ci][p][c*CH+s] so each chunk DMA is 128 descriptors x 4-16KB
   contiguous; measured DMA rate rose from ~300 to ~410 GB/s.

4. A ~90-matmul warm-up burst opens the PE HAM clock gate (1.2->2.4GHz)
   before the first real projections; later q-chunk projections are
   emitted one 512-block (4 DoubleRow pair-MMs) per sweep inside the
   attention stream where PE has slack.

Attention machinery unchanged from v2: transposed-score layout, exp on
ScalarE in [128,1024] calls, P^T @ [V | 1] PSUM accumulation with the ones
column producing the softmax denominator (ops), fused DVE normalize, with
sps 3-deep x [128,1024] (6 banks) + ops 2 banks = 8 PSUM banks.
"""

import math
import os

import numpy as np

B, S, D, DK, DV = 8, 4096, 1024, 128, 128
P = 128
SB = 512  # q-block width (attention) and projection block
CH = 1024  # load chunk (sequence cols per stage load)
CD = D // P  # 8 d-chunks
NSB = S // SB  # 8 q-blocks
NKC = S // P  # 32 key chunks
NCH = S // CH  # 4 load chunks per tensor
JPB = SB // P  # 4 q-subchunks per block
SCALE = 1.0 / math.sqrt(DK)

FP8_QK = os.environ.get("FP8_QK", "1") != "0"  # fp8-e4m3 x_q/x_k/Wq/Wk + DoubleRow projections

_cache = {}

# kept for test.py compat (unused)
XBAR_DUAL = False
SWDGE_QUEUES = 1


def _emit(tc, aps):
    from concourse import mybir

    nc = tc.nc
    bf16 = mybir.dt.bfloat16
    f32 = mybir.dt.float32

    qT, kT, vT, wq, wk, wv, bq, bk, bv, out = aps

    out_ap = out.rearrange("(nb j p) d -> nb p j d", p=P, j=JPB)

    from contextlib import ExitStack

    with ExitStack() as ctx:
        consts = ctx.enter_context(tc.tile_pool(name="consts", bufs=1))
        qkv = ctx.enter_context(tc.tile_pool(name="qkv", bufs=1))
        qtp = ctx.enter_context(tc.tile_pool(name="qt", bufs=NSB))
        stagep = ctx.enter_context(tc.tile_pool(name="stage", bufs=6))
        # deep pt pool: one q-block's full stash plus in-flight tiles
        ptp = ctx.enter_context(tc.tile_pool(name="pt", bufs=34))
        outp = ctx.enter_context(tc.tile_pool(name="outp", bufs=3))
        smallp = ctx.enter_context(tc.tile_pool(name="small", bufs=8))
        psump = ctx.enter_context(tc.tile_pool(name="ps", bufs=2, space="PSUM"))

        # --- constants ---
        if FP8_QK:
            f8 = mybir.dt.float8e4
            wq_sb = consts.tile([P, CD // 2, 2, DK], f8)
            wk_sb = consts.tile([P, CD // 2, 2, DK], f8)
        else:
            wq_sb = consts.tile([P, CD, DK], bf16)
            wk_sb = consts.tile([P, CD, DK], bf16)
        wv_sb = consts.tile([P, CD, DV], bf16)
        # all weights/biases on gpsimd SWDGE; wk first (first projections)
        nc.gpsimd.dma_start(out=wk_sb, in_=wk)
        nc.gpsimd.dma_start(out=wv_sb, in_=wv)
        nc.gpsimd.dma_start(out=wq_sb, in_=wq)
        bq_sb = consts.tile([P, 1], f32)
        bk_sb = consts.tile([P, 1], f32)
        bv_sb = consts.tile([P, DV], f32)
        nc.gpsimd.dma_start(out=bq_sb, in_=bq)
        nc.gpsimd.dma_start(out=bk_sb, in_=bk)
        nc.gpsimd.dma_start(out=bv_sb, in_=bv)

        warm_in = consts.tile([P, 8], f32)
        warm_out = consts.tile([P, 8], f32)
        nc.vector.memset(warm_in, 0.0)

        # persistent per-core tensors
        kt_sb = qkv.tile([P, S], bf16)  # K^T  [dk, s]
        vp_sb = qkv.tile([P, NKC, DV + 1], bf16)  # V' natural [s%128, chunk, dv+1]
        nc.vector.memset(vp_sb[:, :, DV : DV + 1], 1.0)
        qts = [qtp.tile([P, SB], bf16, tag="qt", name=f"qt{i}") for i in range(NSB)]

        def load_chunk(src_ap, ci, nm, dt, quarters=False, eng=None):
            # c-splits let the projections' subtile deps fire as each split
            # lands.  eng picks the HWDGE ring: per-DMA fixed overheads
            # serialize per ring, so the front splits across sync AND scalar
            # (ScalarE's FIFO is ACT-free until the first exp).
            st = stagep.tile([P, CD, CH], dt, tag="stage", name=f"st_{nm}{ci}")
            sl = src_ap[ci]
            step = CD // 4 if quarters else CD // 2
            for c0 in range(0, CD, step):
                e = eng if eng is not None else nc.sync
                e.dma_start(out=st[:, c0 : c0 + step, :], in_=sl[:, c0 : c0 + step, :])
            return st

        def load_chunk_dual(src_ap, ci, nm, dt):
            # halves on both rings in parallel
            st = stagep.tile([P, CD, CH], dt, tag="stage", name=f"st_{nm}{ci}")
            sl = src_ap[ci]
            h = CD // 2
            nc.sync.dma_start(out=st[:, 0:h, :], in_=sl[:, 0:h, :])
            nc.scalar.dma_start(out=st[:, h:CD, :], in_=sl[:, h:CD, :])
            return st

        def project_x(st, ci, w_sb, b_sb, dst, h_lo, h_hi, nm):
            # dst: callable sb -> (out_ap) receiving the biased block
            for h in range(h_lo, h_hi):
                sb = ci * (CH // SB) + h
                ps = psump.tile([P, SB], f32, tag="sps", bufs=3, name=f"{nm}ps{sb}")
                if FP8_QK:
                    for c2 in range(CD // 2):
                        nc.tensor.matmul(
                            ps,
                            w_sb[:, c2, :, :],
                            st[:, 2 * c2 : 2 * c2 + 2, h * SB : (h + 1) * SB],
                            start=(c2 == 0),
                            stop=(c2 == CD // 2 - 1),
                            perf_mode=mybir.MatmulPerfMode.DoubleRow,
                        )
                else:
                    for c in range(CD):
                        nc.tensor.matmul(
                            ps,
                            w_sb[:, c, :],
                            st[:, c, h * SB : (h + 1) * SB],
                            start=(c == 0),
                            stop=(c == CD - 1),
                        )
                nc.vector.tensor_scalar_add(dst(sb), ps, b_sb)

        def project_kt(st, ci, h_lo=0, h_hi=CH // SB):
            project_x(
                st, ci, wk_sb, bk_sb,
                lambda sb: kt_sb[:, sb * SB : (sb + 1) * SB], h_lo, h_hi, "k",
            )

        def project_qt(st, ci, h_lo=0, h_hi=CH // SB):
            project_x(st, ci, wq_sb, bq_sb, lambda sb: qts[sb], h_lo, h_hi, "q")

        def project_v(st, ci, j_lo=0, j_hi=CH // P):
            # 4 j-outputs packed per [128, 512] psum tile; start=True only on
            # the first matmul per bank (whole-bank has_written clear), later
            # j's first write overwrites on cleared bits.
            for j4 in range(j_lo, j_hi, 4):
                vps = psump.tile(
                    [P, SB], f32, tag="sps", bufs=3, name=f"vps{ci}_{j4}"
                )
                for j in range(j4, j4 + 4):
                    jj = j - j4
                    for c in range(CD):
                        nc.tensor.matmul(
                            vps[:, jj * DV : (jj + 1) * DV],
                            st[:, c, j * P : (j + 1) * P],
                            wv_sb[:, c, :],
                            start=(jj == 0 and c == 0),
                            stop=(c == CD - 1),
                        )
                kk0 = ci * (CH // P) + j4
                nc.vector.tensor_copy(
                    vp_sb[:, kk0 : kk0 + 4, 0:DV],
                    vps.rearrange("p (j d) -> p j d", j=4),
                )

        # --- attention emission helpers ---
        # key chunks grouped 2 per exp call ([128, 1024] ACTIVATEs); the
        # smaller group buys a 3-deep sps rotation (3x2=6 banks) that
        # decouples the scores matmuls from exp WAR jitter.
        #
        # Deferred-PV pipeline: attn_scores stashes each group's exp'd
        # probabilities (pt) in a deep SBUF pool; attn_pv consumes them
        # later - each q-block's PV matmuls ride the NEXT q-block's
        # scores/exp stream.  The ScalarE ACT chain then depends only on
        # scores matmuls (never on PV/ops/DVE), and two q-blocks' worth of
        # exp fits inside the DMA-bound load front with only 2 ops banks.
        groups = [(g * 2, 2) for g in range(NKC // 2)]
        ngrp = len(groups)
        qb_ops = {}
        pt_stash = {}

        def attn_begin(qb):
            opsA = psump.tile([P, 2, DV + 1], f32, tag="ops", bufs=2, name=f"opsA{qb}")
            opsB = psump.tile([P, 2, DV + 1], f32, tag="ops", bufs=2, name=f"opsB{qb}")
            qb_ops[qb] = [opsA[:, 0, :], opsA[:, 1, :], opsB[:, 0, :], opsB[:, 1, :]]

        def attn_scores(qb, g_lo, g_hi):
            for gi in range(g_lo, g_hi):
                k0, gn = groups[gi]
                sps = psump.tile(
                    [P, gn * SB], f32, tag="sps", bufs=3, name=f"sps{qb}_{gi}"
                )
                for h in range(gn):
                    kk = k0 + h
                    nc.tensor.matmul(
                        sps[:, h * SB : (h + 1) * SB],
                        kt_sb[:, kk * P : (kk + 1) * P],
                        qts[qb],
                        start=True,
                        stop=True,
                    )
                pt = ptp.tile([P, gn * SB], bf16, tag="pt", name=f"pt{qb}_{gi}")
                nc.scalar.activation(
                    pt, sps, mybir.ActivationFunctionType.Exp, scale=SCALE
                )
                pt_stash[(qb, gi)] = pt

        def attn_pv(qb, g_lo, g_hi):
            ops = qb_ops[qb]
            for gi in range(g_lo, g_hi):
                k0, gn = groups[gi]
                pt = pt_stash.pop((qb, gi))
                for h in range(gn):
                    kk = k0 + h
                    for j in range(JPB):
                        # start=True clears has_written for the WHOLE bank, so
                        # only the first matmul per bank (j=0 / j=2) may set it;
                        # the partner tile's first write lands on cleared bits
                        # and overwrites (per-element has_written semantics).
                        nc.tensor.matmul(
                            ops[j],
                            pt[:, h * SB + j * P : h * SB + (j + 1) * P],
                            vp_sb[:, kk, :],
                            start=(gi == 0 and h == 0 and j % 2 == 0),
                            stop=(gi == ngrp - 1 and h == gn - 1),
                        )

        def attn_end(qb):
            ops = qb_ops.pop(qb)
            ostage = outp.tile([P, JPB, DV], f32, tag="ostage", name=f"ostage{qb}")
            for j in range(JPB):
                recip = smallp.tile([P, 1], f32, tag="recip", name=f"recip{qb}_{j}")
                nc.vector.reciprocal(recip, ops[j][:, DV : DV + 1])
                nc.vector.scalar_tensor_tensor(
                    ostage[:, j, :],
                    ops[j][:, 0:DV],
                    recip,
                    bv_sb,
                    mybir.AluOpType.mult,
                    mybir.AluOpType.add,
                )
                if j % 2 == 1:  # store halves as they complete (shorter tail);
                    nc.sync.dma_start(
                        out=out_ap[qb][:, j - 1 : j + 1, :],
                        in_=ostage[:, j - 1 : j + 1, :],
                    )

        # Global scores / deferred-PV streams.  The PV stream trails the
        # scores stream by LAG groups across the WHOLE kernel, so (a) PV MMs
        # at the PE-FIFO head are never waiting on a fresh exp or on the
        # attn_end DVE chain, (b) there is no PV drain lump at q-block
        # boundaries or at kernel end (PV finishes LAG groups after the last
        # ACT), and (c) attn_end/attn_begin ride mid-stream.
        LAG = 3
        sc_pos = {"n": 0}
        pv_pos = {"n": 0}
        sc_prog = {}  # qb -> groups of scores emitted

        def pv_advance_one(drain=False):
            qb, g = divmod(pv_pos["n"], ngrp)
            if qb >= NSB or (qb, g) not in pt_stash:
                return False  # stream exhausted / that exp not emitted yet
            need = g + 1 + (LAG if g > 0 else 2 * LAG)
            if not drain and sc_prog.get(qb, 0) < min(need, ngrp):
                # keep LAG groups between a qb's ACTs and its PV; double at
                # the q-block boundary so the previous block's attn_end DVE
                # chain finishes before this block's first (ops-WAR-gated) PV
                return False
            attn_pv(qb, g, g + 1)
            pv_pos["n"] += 1
            if g + 1 == ngrp:
                attn_end(qb)
                if qb + 1 < NSB:
                    attn_begin(qb + 1)
            return True

        def sc(qb, g):
            attn_scores(qb, g, g + 1)
            sc_pos["n"] += 1
            sc_prog[qb] = g + 1
            while pv_pos["n"] < sc_pos["n"] - LAG:
                if not pv_advance_one():
                    break

        # --- software pipeline in emission order (engines run their streams
        # FIFO, so emission order IS the per-engine execution order) ---
        kdt = mybir.dt.float8e4 if FP8_QK else bf16
        # k0 triggers are ScalarE's first instructions (ring otherwise idle
        # until the first exp); q0 leads the sync ring - both rings stream
        # their first chunk concurrently, halving the pre-first-exp startup
        stk0 = load_chunk(kT, 0, "k", kdt, quarters=True, eng=nc.scalar)
        stq0 = load_chunk(qT, 0, "q", kdt, quarters=True)
        stv0 = load_chunk(vT, 0, "v", bf16, quarters=True)
        # warm the exp table set while loads stream
        nc.scalar.activation(warm_out, warm_in, mybir.ActivationFunctionType.Exp)

        # PE warm-up burst: dependency-free matmuls open the HAM clock-gate
        # (1.2 -> 2.4 GHz) before the first real projections.
        warm_w = consts.tile([P, 64], bf16)
        nc.vector.memset(warm_w, 0.0)
        wps = psump.tile([P, SB], f32, tag="sps", bufs=3, name="warmps")
        for _ in range(90):
            nc.tensor.matmul(wps[0:64, 0:64], warm_w, warm_w, start=True, stop=True)

        # first chunk's projections at block granularity so the first
        # attention groups fire as soon as kc 0..3 are projected
        project_kt(stk0, 0, 0, 1)  # kc 0..3
        project_qt(stq0, 0)  # qt[0], qt[1]
        project_v(stv0, 0, 0, 4)  # vp 0..3

        # Load front: qb0's, qb1's and (trailing 6 groups) qb2's scores+exp
        # all ride the DMA-bound window - 48 of the 128 ACTs complete before
        # the steady state begins, soaking up the chunk-wait gaps.
        attn_begin(0)
        for g in range(0, 2):  # kc 0..3
            sc(0, g)
        project_kt(stk0, 0, 1, 2)  # kc 4..7
        project_v(stv0, 0, 4, 8)  # vp 4..7
        stk = load_chunk(kT, 1, "k", kdt, eng=nc.scalar)
        stv = load_chunk(vT, 1, "v", bf16)
        for g in range(2, 4):  # kc 4..7
            sc(0, g)
        project_kt(stk, 1)
        project_v(stv, 1)
        stq1 = load_chunk(qT, 1, "q", kdt)
        stk = load_chunk(kT, 2, "k", kdt, eng=nc.scalar)
        stv = load_chunk(vT, 2, "v", bf16)
        for g in range(4, 6):  # kc 8..11, needs chunk 1
            sc(0, g)
        project_qt(stq1, 1)  # qts[2], qts[3]
        for g in range(6, 8):
            sc(0, g)
        project_kt(stk, 2)
        project_v(stv, 2)
        stk = load_chunk(kT, 3, "k", kdt, eng=nc.scalar)
        stv = load_chunk(vT, 3, "v", bf16)
        for g in range(8, 12):  # kc 16..23, needs chunk 2
            sc(0, g)
        project_kt(stk, 3)
        project_v(stv, 3)
        stq2 = load_chunk(qT, 2, "q", kdt)
        for g in range(12, ngrp):  # kc 24..31
            sc(0, g)
        stq3 = load_chunk(qT, 3, "q", kdt)  # before out-stores clog the ring

        # steady state: one continuous scores sweep per remaining q-block;
        # the deferred-PV stream self-advances LAG groups behind; one
        # qt-projection 512-block (4 DoubleRow pair-MMs, ~1us PE) inserted
        # per sweep.
        for qs in range(1, NSB):
            for g in range(ngrp):
                sc(qs, g)
                if g == 3:
                    if qs == 3:
                        project_qt(stq2, 2, 0, 1)  # qts[4]
                    elif qs == 4:
                        project_qt(stq2, 2, 1, 2)  # qts[5]
                    elif qs == 5:
                        project_qt(stq3, 3, 0, 1)  # qts[6]
                    elif qs == 6:
                        project_qt(stq3, 3, 1, 2)  # qts[7]
        while pv_advance_one(drain=True):
            pass


def build(s_len=S):
    import concourse.tile as tile
    from concourse import bacc, mybir

    nc = bacc.Bacc(
        "TRN2",
        target_bir_lowering=False,
        debug=False,
        enable_asserts=False,
        num_devices=8,
    )
    f32 = mybir.dt.float32
    bf16 = mybir.dt.bfloat16
    kdt = mybir.dt.float8e4 if FP8_QK else bf16
    if FP8_QK:
        wshape = [P, CD // 2, 2, DK]
        wdt = mybir.dt.float8e4
    else:
        wshape = [P, CD, DK]
        wdt = bf16
    aps = [
        nc.dram_tensor("qT", [NCH, P, CD, CH], kdt, kind="ExternalInput").ap(),
        nc.dram_tensor("kT", [NCH, P, CD, CH], kdt, kind="ExternalInput").ap(),
        nc.dram_tensor("vT", [NCH, P, CD, CH], bf16, kind="ExternalInput").ap(),
        nc.dram_tensor("Wq", wshape, wdt, kind="ExternalInput").ap(),
        nc.dram_tensor("Wk", wshape, wdt, kind="ExternalInput").ap(),
        nc.dram_tensor("Wv", [P, CD, DV], bf16, kind="ExternalInput").ap(),
        nc.dram_tensor("bq", [DK, 1], f32, kind="ExternalInput").ap(),
        nc.dram_tensor("bk", [DK, 1], f32, kind="ExternalInput").ap(),
        nc.dram_tensor("bv", [P, DV], f32, kind="ExternalInput").ap(),
        nc.dram_tensor("out", [S, DV], f32, kind="ExternalOutput").ap(),
    ]
    with tile.TileContext(nc) as tc:
        _emit(tc, aps)
    nc.compile()
    return nc


def make_in_maps(inputs, s_len=S):
    import ml_dtypes

    bf = ml_dtypes.bfloat16
    f8 = ml_dtypes.float8_e4m3
    kdt = f8 if FP8_QK else bf

    def prep_w(w):
        # [d, k] -> [p, c, k] with d = c*128 + p
        w = np.asarray(w, np.float32).reshape(CD, P, -1).transpose(1, 0, 2)
        return np.ascontiguousarray(w).astype(bf)

    def prep_w_pair(w):
        # [d, k] -> [p, c2, i, k] with d = (2*c2 + i)*128 + p
        w = np.asarray(w, np.float32).reshape(CD // 2, 2, P, -1).transpose(2, 0, 1, 3)
        return np.ascontiguousarray(w).astype(f8)

    prep_wqk = prep_w_pair if FP8_QK else prep_w

    weights = {
        "Wq": prep_wqk(inputs["Wq"]),
        "Wk": prep_wqk(inputs["Wk"]),
        "Wv": prep_w(inputs["Wv"]),
        "bq": np.ascontiguousarray(inputs["bq"], dtype=np.float32).reshape(DK, 1),
        "bk": np.ascontiguousarray(inputs["bk"], dtype=np.float32).reshape(DK, 1),
        "bv": np.ascontiguousarray(
            np.broadcast_to(
                np.asarray(inputs["bv"], np.float32).reshape(1, DV), (P, DV)
            )
        ),
    }

    def prep_x(x, dt):
        # [s, d] f32 -> [ci, p, c, s_local] chunk-contiguous staging
        x = np.asarray(x, np.float32).reshape(NCH, CH, CD, P).transpose(0, 3, 2, 1)
        return x.astype(dt)

    in_maps = []
    for i in range(B):
        m = dict(weights)
        m["qT"] = prep_x(inputs["query"][i], kdt)
        m["kT"] = prep_x(inputs["key"][i], kdt)
        m["vT"] = prep_x(inputs["value"][i], bf)
        in_maps.append(m)
    return in_maps


def kernel(**inputs):
    from concourse.bass_utils import run_bass_kernel_spmd

    if "nc" not in _cache:
        _cache["nc"] = build(S)
    nc = _cache["nc"]
    in_maps = make_in_maps(inputs, S)
    res = run_bass_kernel_spmd(nc, in_maps, core_ids=list(range(B)))
    return np.stack([r["out"] for r in res.results], axis=0)


# revision 41
# speedup vs baseline: 1.0056x; 1.0056x over previous
"""Single-head attention (B=8, S=4096, D=1024, DK=DV=128) on 8 TRN2 NeuronCores.

Sharding: data-parallel over batch - one batch element per core, the three
Linear weights replicated. No collectives.

Measured (core 0 traced, 8 cores running): ~185-190us at the chip's
nominal clock (the chip's P0 power state adds up to ~20% run-to-run),
rel_err 1.10e-2 vs the f32 reference (tolerance 2e-2); v2 baseline was
212-221us at rel_err 1.7e-3.

Trace-driven changes from the v2 baseline:

1. ScalarE is the steady-state bottleneck: each [128,1024] exp ACTIVATE
   costs (N+352)/1.2 ns and the per-call overhead never pipelines away,
   so 128 ACTs = ~129us effective minimum. Everything else is scheduled
   around keeping that ACT stream dense:
   - all x-chunk load DMA triggers moved off the scalar queue onto the
     sync HWDGE ring only (one ring's DMA already fans out to all 16 SDMA
     engines; sequential c-splits keep projection subtile deps early);
     weights/biases on gpsimd SWDGE.
   - deferred-PV pipeline: each group's exp output (pt) is stashed in a
     deep SBUF pool (34 tiles) and its P^T@[V|1] matmuls are emitted LAG=3
     groups later (2*LAG at q-block boundaries, covering the attn_end DVE
     chain), so the PE-FIFO head never waits on a fresh ACT or on DVE and
     the ACT chain depends only on scores matmuls. Steady state measures
     ~1000ns/ACT (the floor).
   - qb0's scores+exp ride the DMA-bound load front SOLO, so attn_end(0)
     fires right at front-end and qb1's deferred PV (ops-WAR-gated on it)
     overlaps qb1's own gapless exp sweep instead of piling up behind a
     late attn_end (interleaving qb0+qb1 in the front was measured worse:
     it pushes attn_end(0) ~16us later and stalls ScalarE ~6us).

2. FP8_QK: query/key activations AND Wq/Wk are cast host-side to fp8-e4m3
   (TRN float8e4; values well under the 240 max). The Q/K projections run
   as fp8 DoubleRow matmuls (contraction pairs of d-chunks, 2 MACs/cycle):
   4 pair-MMs per 512-block instead of 8, halving Q/K projection PE time,
   and the Q/K input DMA bytes halve (the front is HBM-bound at ~300-410
   GB/s/core, so 16MB instead of 24MB directly shortens it). V path stays
   bf16: the output is a near-uniform softmax average, so V-side
   quantization error does not average out (numpy-sim: V-fp8 1.9e-2 vs
   QK-fp8 1.1e-2, both calibrated within 1% of HW).

3. Host staging is chunk-contiguous: x^T is pre-arranged per load-chunk as
   [ci][p][c*CH+s] so each chunk DMA is 128 descriptors x 4-16KB
   contiguous; measured DMA rate rose from ~300 to ~410 GB/s.

4. A ~90-matmul warm-up burst opens the PE HAM clock gate (1.2->2.4GHz)
   before the first real projections; later q-chunk projections are
   emitted one 512-block (4 DoubleRow pair-MMs) per sweep inside the
   attention stream where PE has slack.

Attention machinery unchanged from v2: transposed-score layout, exp on
ScalarE in [128,1024] calls, P^T @ [V | 1] PSUM accumulation with the ones
column producing the softmax denominator (ops), fused DVE normalize, with
sps 3-deep x [128,1024] (6 banks) + ops 2 banks = 8 PSUM banks.
"""

import math
import os

import numpy as np

B, S, D, DK, DV = 8, 4096, 1024, 128, 128
P = 128
SB = 512  # q-block width (attention) and projection block
CH = 1024  # load chunk (sequence cols per stage load)
CD = D // P  # 8 d-chunks
NSB = S // SB  # 8 q-blocks
NKC = S // P  # 32 key chunks
NCH = S // CH  # 4 load chunks per tensor
JPB = SB // P  # 4 q-subchunks per block
SCALE = 1.0 / math.sqrt(DK)

FP8_QK = os.environ.get("FP8_QK", "1") != "0"  # fp8-e4m3 x_q/x_k/Wq/Wk + DoubleRow projections

_cache = {}

# kept for test.py compat (unused)
XBAR_DUAL = False
SWDGE_QUEUES = 1


def _emit(tc, aps):
    from concourse import mybir

    nc = tc.nc
    bf16 = mybir.dt.bfloat16
    f32 = mybir.dt.float32

    qT, kT, vT, wq, wk, wv, bq, bk, bv, out = aps

    out_ap = out.rearrange("(nb j p) d -> nb p j d", p=P, j=JPB)

    from contextlib import ExitStack

    with ExitStack() as ctx:
        consts = ctx.enter_context(tc.tile_pool(name="consts", bufs=1))
        qkv = ctx.enter_context(tc.tile_pool(name="qkv", bufs=1))
        qtp = ctx.enter_context(tc.tile_pool(name="qt", bufs=NSB))
        stagep = ctx.enter_context(tc.tile_pool(name="stage", bufs=6))
        # deep pt pool: one q-block's full stash plus in-flight tiles
        ptp = ctx.enter_context(tc.tile_pool(name="pt", bufs=34))
        outp = ctx.enter_context(tc.tile_pool(name="outp", bufs=3))
        smallp = ctx.enter_context(tc.tile_pool(name="small", bufs=8))
        psump = ctx.enter_context(tc.tile_pool(name="ps", bufs=2, space="PSUM"))

        # --- constants ---
        if FP8_QK:
            f8 = mybir.dt.float8e4
            wq_sb = consts.tile([P, CD // 2, 2, DK], f8)
            wk_sb = consts.tile([P, CD // 2, 2, DK], f8)
        else:
            wq_sb = consts.tile([P, CD, DK], bf16)
            wk_sb = consts.tile([P, CD, DK], bf16)
        wv_sb = consts.tile([P, CD, DV], bf16)
        # wk first on sync (needed by the first projections), rest on gpsimd
        nc.sync.dma_start(out=wk_sb, in_=wk)
        nc.gpsimd.dma_start(out=wv_sb, in_=wv)
        nc.gpsimd.dma_start(out=wq_sb, in_=wq)
        bq_sb = consts.tile([P, 1], f32)
        bk_sb = consts.tile([P, 1], f32)
        bv_sb = consts.tile([P, DV], f32)
        nc.gpsimd.dma_start(out=bq_sb, in_=bq)
        nc.gpsimd.dma_start(out=bk_sb, in_=bk)
        nc.gpsimd.dma_start(out=bv_sb, in_=bv)

        # warm the exp table set while loads stream
        warm_in = consts.tile([P, 8], f32)
        warm_out = consts.tile([P, 8], f32)
        nc.vector.memset(warm_in, 0.0)
        nc.scalar.activation(warm_out, warm_in, mybir.ActivationFunctionType.Exp)

        # persistent per-core tensors
        kt_sb = qkv.tile([P, S], bf16)  # K^T  [dk, s]
        vp_sb = qkv.tile([P, NKC, DV + 1], bf16)  # V' natural [s%128, chunk, dv+1]
        nc.vector.memset(vp_sb[:, :, DV : DV + 1], 1.0)
        qts = [qtp.tile([P, SB], bf16, tag="qt", name=f"qt{i}") for i in range(NSB)]

        def load_chunk(src_ap, ci, nm, dt, quarters=False, eng=None):
            # c-splits let the projections' subtile deps fire as each split
            # lands.  eng picks the HWDGE ring: per-DMA fixed overheads
            # serialize per ring, so the front splits across sync AND scalar
            # (ScalarE's FIFO is ACT-free until the first exp).
            st = stagep.tile([P, CD, CH], dt, tag="stage", name=f"st_{nm}{ci}")
            sl = src_ap[ci]
            step = CD // 4 if quarters else CD // 2
            for c0 in range(0, CD, step):
                e = eng if eng is not None else nc.sync
                e.dma_start(out=st[:, c0 : c0 + step, :], in_=sl[:, c0 : c0 + step, :])
            return st

        def load_chunk_dual(src_ap, ci, nm, dt):
            # halves on both rings in parallel
            st = stagep.tile([P, CD, CH], dt, tag="stage", name=f"st_{nm}{ci}")
            sl = src_ap[ci]
            h = CD // 2
            nc.sync.dma_start(out=st[:, 0:h, :], in_=sl[:, 0:h, :])
            nc.scalar.dma_start(out=st[:, h:CD, :], in_=sl[:, h:CD, :])
            return st

        def project_x(st, ci, w_sb, b_sb, dst, h_lo, h_hi, nm):
            # dst: callable sb -> (out_ap) receiving the biased block
            for h in range(h_lo, h_hi):
                sb = ci * (CH // SB) + h
                ps = psump.tile([P, SB], f32, tag="sps", bufs=3, name=f"{nm}ps{sb}")
                if FP8_QK:
                    for c2 in range(CD // 2):
                        nc.tensor.matmul(
                            ps,
                            w_sb[:, c2, :, :],
                            st[:, 2 * c2 : 2 * c2 + 2, h * SB : (h + 1) * SB],
                            start=(c2 == 0),
                            stop=(c2 == CD // 2 - 1),
                            perf_mode=mybir.MatmulPerfMode.DoubleRow,
                        )
                else:
                    for c in range(CD):
                        nc.tensor.matmul(
                            ps,
                            w_sb[:, c, :],
                            st[:, c, h * SB : (h + 1) * SB],
                            start=(c == 0),
                            stop=(c == CD - 1),
                        )
                nc.vector.tensor_scalar_add(dst(sb), ps, b_sb)

        def project_kt(st, ci, h_lo=0, h_hi=CH // SB):
            project_x(
                st, ci, wk_sb, bk_sb,
                lambda sb: kt_sb[:, sb * SB : (sb + 1) * SB], h_lo, h_hi, "k",
            )

        def project_qt(st, ci, h_lo=0, h_hi=CH // SB):
            project_x(st, ci, wq_sb, bq_sb, lambda sb: qts[sb], h_lo, h_hi, "q")

        def project_v(st, ci, j_lo=0, j_hi=CH // P):
            # 4 j-outputs packed per [128, 512] psum tile; start=True only on
            # the first matmul per bank (whole-bank has_written clear), later
            # j's first write overwrites on cleared bits.
            for j4 in range(j_lo, j_hi, 4):
                vps = psump.tile(
                    [P, SB], f32, tag="sps", bufs=3, name=f"vps{ci}_{j4}"
                )
                for j in range(j4, j4 + 4):
                    jj = j - j4
                    for c in range(CD):
                        nc.tensor.matmul(
                            vps[:, jj * DV : (jj + 1) * DV],
                            st[:, c, j * P : (j + 1) * P],
                            wv_sb[:, c, :],
                            start=(jj == 0 and c == 0),
                            stop=(c == CD - 1),
                        )
                kk0 = ci * (CH // P) + j4
                nc.vector.tensor_copy(
                    vp_sb[:, kk0 : kk0 + 4, 0:DV],
                    vps.rearrange("p (j d) -> p j d", j=4),
                )

        # --- attention emission helpers ---
        # key chunks grouped 2 per exp call ([128, 1024] ACTIVATEs); the
        # smaller group buys a 3-deep sps rotation (3x2=6 banks) that
        # decouples the scores matmuls from exp WAR jitter.
        #
        # Deferred-PV pipeline: attn_scores stashes each group's exp'd
        # probabilities (pt) in a deep SBUF pool; attn_pv consumes them
        # later - each q-block's PV matmuls ride the NEXT q-block's
        # scores/exp stream.  The ScalarE ACT chain then depends only on
        # scores matmuls (never on PV/ops/DVE), and two q-blocks' worth of
        # exp fits inside the DMA-bound load front with only 2 ops banks.
        groups = [(g * 2, 2) for g in range(NKC // 2)]
        ngrp = len(groups)
        qb_ops = {}
        pt_stash = {}

        def attn_begin(qb):
            opsA = psump.tile([P, 2, DV + 1], f32, tag="ops", bufs=2, name=f"opsA{qb}")
            opsB = psump.tile([P, 2, DV + 1], f32, tag="ops", bufs=2, name=f"opsB{qb}")
            qb_ops[qb] = [opsA[:, 0, :], opsA[:, 1, :], opsB[:, 0, :], opsB[:, 1, :]]

        def attn_scores(qb, g_lo, g_hi):
            for gi in range(g_lo, g_hi):
                k0, gn = groups[gi]
                sps = psump.tile(
                    [P, gn * SB], f32, tag="sps", bufs=3, name=f"sps{qb}_{gi}"
                )
                for h in range(gn):
                    kk = k0 + h
                    nc.tensor.matmul(
                        sps[:, h * SB : (h + 1) * SB],
                        kt_sb[:, kk * P : (kk + 1) * P],
                        qts[qb],
                        start=True,
                        stop=True,
                    )
                pt = ptp.tile([P, gn * SB], bf16, tag="pt", name=f"pt{qb}_{gi}")
                nc.scalar.activation(
                    pt, sps, mybir.ActivationFunctionType.Exp, scale=SCALE
                )
                pt_stash[(qb, gi)] = pt

        def attn_pv(qb, g_lo, g_hi):
            ops = qb_ops[qb]
            for gi in range(g_lo, g_hi):
                k0, gn = groups[gi]
                pt = pt_stash.pop((qb, gi))
                for h in range(gn):
                    kk = k0 + h
                    for j in range(JPB):
                        # start=True clears has_written for the WHOLE bank, so
                        # only the first matmul per bank (j=0 / j=2) may set it;
                        # the partner tile's first write lands on cleared bits
                        # and overwrites (per-element has_written semantics).
                        nc.tensor.matmul(
                            ops[j],
                            pt[:, h * SB + j * P : h * SB + (j + 1) * P],
                            vp_sb[:, kk, :],
                            start=(gi == 0 and h == 0 and j % 2 == 0),
                            stop=(gi == ngrp - 1 and h == gn - 1),
                        )

        def attn_end(qb):
            ops = qb_ops.pop(qb)
            ostage = outp.tile([P, JPB, DV], f32, tag="ostage", name=f"ostage{qb}")
            for j in range(JPB):
                recip = smallp.tile([P, 1], f32, tag="recip", name=f"recip{qb}_{j}")
                nc.vector.reciprocal(recip, ops[j][:, DV : DV + 1])
                nc.vector.scalar_tensor_tensor(
                    ostage[:, j, :],
                    ops[j][:, 0:DV],
                    recip,
                    bv_sb,
                    mybir.AluOpType.mult,
                    mybir.AluOpType.add,
                )
                if j % 2 == 1:  # store halves as they complete (shorter tail);
                    nc.sync.dma_start(
                        out=out_ap[qb][:, j - 1 : j + 1, :],
                        in_=ostage[:, j - 1 : j + 1, :],
                    )

        # Global scores / deferred-PV streams.  The PV stream trails the
        # scores stream by LAG groups across the WHOLE kernel, so (a) PV MMs
        # at the PE-FIFO head are never waiting on a fresh exp or on the
        # attn_end DVE chain, (b) there is no PV drain lump at q-block
        # boundaries or at kernel end (PV finishes LAG groups after the last
        # ACT), and (c) attn_end/attn_begin ride mid-stream.
        LAG = 3
        sc_pos = {"n": 0}
        pv_pos = {"n": 0}
        sc_prog = {}  # qb -> groups of scores emitted

        def pv_advance_one(drain=False):
            qb, g = divmod(pv_pos["n"], ngrp)
            if qb >= NSB or (qb, g) not in pt_stash:
                return False  # stream exhausted / that exp not emitted yet
            need = g + 1 + (LAG if g > 0 else 2 * LAG)
            if not drain and sc_prog.get(qb, 0) < min(need, ngrp):
                # keep LAG groups between a qb's ACTs and its PV; double at
                # the q-block boundary so the previous block's attn_end DVE
                # chain finishes before this block's first (ops-WAR-gated) PV
                return False
            attn_pv(qb, g, g + 1)
            pv_pos["n"] += 1
            if g + 1 == ngrp:
                attn_end(qb)
                if qb + 1 < NSB:
                    attn_begin(qb + 1)
            return True

        def sc(qb, g):
            attn_scores(qb, g, g + 1)
            sc_pos["n"] += 1
            sc_prog[qb] = g + 1
            while pv_pos["n"] < sc_pos["n"] - LAG:
                if not pv_advance_one():
                    break

        # --- software pipeline in emission order (engines run their streams
        # FIFO, so emission order IS the per-engine execution order) ---
        kdt = mybir.dt.float8e4 if FP8_QK else bf16
        stk0 = load_chunk(kT, 0, "k", kdt, quarters=True)
        stq0 = load_chunk(qT, 0, "q", kdt, quarters=True)
        stv0 = load_chunk(vT, 0, "v", bf16, quarters=True)

        # PE warm-up burst: dependency-free matmuls open the HAM clock-gate
        # (1.2 -> 2.4 GHz) before the first real projections.
        warm_w = consts.tile([P, 64], bf16)
        nc.vector.memset(warm_w, 0.0)
        wps = psump.tile([P, SB], f32, tag="sps", bufs=3, name="warmps")
        for _ in range(90):
            nc.tensor.matmul(wps[0:64, 0:64], warm_w, warm_w, start=True, stop=True)

        # first chunk's projections at block granularity so the first
        # attention groups fire as soon as kc 0..3 are projected
        project_kt(stk0, 0, 0, 1)  # kc 0..3
        project_qt(stq0, 0)  # qt[0], qt[1]
        project_v(stv0, 0, 0, 4)  # vp 0..3

        # Load front: qb0's, qb1's and (trailing 6 groups) qb2's scores+exp
        # all ride the DMA-bound window - 48 of the 128 ACTs complete before
        # the steady state begins, soaking up the chunk-wait gaps.
        attn_begin(0)
        for g in range(0, 2):  # kc 0..3
            sc(0, g)
        project_kt(stk0, 0, 1, 2)  # kc 4..7
        project_v(stv0, 0, 4, 8)  # vp 4..7
        stk = load_chunk(kT, 1, "k", kdt)
        stv = load_chunk(vT, 1, "v", bf16)
        for g in range(2, 4):  # kc 4..7
            sc(0, g)
        project_kt(stk, 1)
        project_v(stv, 1)
        stq1 = load_chunk(qT, 1, "q", kdt)
        stk = load_chunk(kT, 2, "k", kdt)
        stv = load_chunk(vT, 2, "v", bf16)
        for g in range(4, 6):  # kc 8..11, needs chunk 1
            sc(0, g)
        project_qt(stq1, 1)  # qts[2], qts[3]
        for g in range(6, 8):
            sc(0, g)
        project_kt(stk, 2)
        project_v(stv, 2)
        stk = load_chunk(kT, 3, "k", kdt)
        stv = load_chunk(vT, 3, "v", bf16)
        for g in range(8, 12):  # kc 16..23, needs chunk 2
            sc(0, g)
        project_kt(stk, 3)
        project_v(stv, 3)
        stq2 = load_chunk(qT, 2, "q", kdt)
        for g in range(12, ngrp):  # kc 24..31
            sc(0, g)
        stq3 = load_chunk(qT, 3, "q", kdt)  # before out-stores clog the ring

        # steady state: one continuous scores sweep per remaining q-block;
        # the deferred-PV stream self-advances LAG groups behind; one
        # qt-projection 512-block (4 DoubleRow pair-MMs, ~1us PE) inserted
        # per sweep.
        for qs in range(1, NSB):
            for g in range(ngrp):
                sc(qs, g)
                if g == 3:
                    if qs == 3:
                        project_qt(stq2, 2, 0, 1)  # qts[4]
                    elif qs == 4:
                        project_qt(stq2, 2, 1, 2)  # qts[5]
                    elif qs == 5:
                        project_qt(stq3, 3, 0, 1)  # qts[6]
                    elif qs == 6:
                        project_qt(stq3, 3, 1, 2)  # qts[7]
        while pv_advance_one(drain=True):
            pass


def build(s_len=S):
    import concourse.tile as tile
    from concourse import bacc, mybir

    nc = bacc.Bacc(
        "TRN2",
        target_bir_lowering=False,
        debug=False,
        enable_asserts=False,
        num_devices=8,
    )
    f32 = mybir.dt.float32
    bf16 = mybir.dt.bfloat16
    kdt = mybir.dt.float8e4 if FP8_QK else bf16
    if FP8_QK:
        wshape = [P, CD // 2, 2, DK]
        wdt = mybir.dt.float8e4
    else:
        wshape = [P, CD, DK]
        wdt = bf16
    aps = [
        nc.dram_tensor("qT", [NCH, P, CD, CH], kdt, kind="ExternalInput").ap(),
        nc.dram_tensor("kT", [NCH, P, CD, CH], kdt, kind="ExternalInput").ap(),
        nc.dram_tensor("vT", [NCH, P, CD, CH], bf16, kind="ExternalInput").ap(),
        nc.dram_tensor("Wq", wshape, wdt, kind="ExternalInput").ap(),
        nc.dram_tensor("Wk", wshape, wdt, kind="ExternalInput").ap(),
        nc.dram_tensor("Wv", [P, CD, DV], bf16, kind="ExternalInput").ap(),
        nc.dram_tensor("bq", [DK, 1], f32, kind="ExternalInput").ap(),
        nc.dram_tensor("bk", [DK, 1], f32, kind="ExternalInput").ap(),
        nc.dram_tensor("bv", [P, DV], f32, kind="ExternalInput").ap(),
        nc.dram_tensor("out", [S, DV], f32, kind="ExternalOutput").ap(),
    ]
    with tile.TileContext(nc) as tc:
        _emit(tc, aps)
    nc.compile()
    return nc


def make_in_maps(inputs, s_len=S):
    import ml_dtypes

    bf = ml_dtypes.bfloat16
    f8 = ml_dtypes.float8_e4m3
    kdt = f8 if FP8_QK else bf

    def prep_w(w):
        # [d, k] -> [p, c, k] with d = c*128 + p
        w = np.asarray(w, np.float32).reshape(CD, P, -1).transpose(1, 0, 2)
        return np.ascontiguousarray(w).astype(bf)

    def prep_w_pair(w):
        # [d, k] -> [p, c2, i, k] with d = (2*c2 + i)*128 + p
        w = np.asarray(w, np.float32).reshape(CD // 2, 2, P, -1).transpose(2, 0, 1, 3)
        return np.ascontiguousarray(w).astype(f8)

    prep_wqk = prep_w_pair if FP8_QK else prep_w

    weights = {
        "Wq": prep_wqk(inputs["Wq"]),
        "Wk": prep_wqk(inputs["Wk"]),
        "Wv": prep_w(inputs["Wv"]),
        "bq": np.ascontiguousarray(inputs["bq"], dtype=np.float32).reshape(DK, 1),
        "bk": np.ascontiguousarray(inputs["bk"], dtype=np.float32).reshape(DK, 1),
        "bv": np.ascontiguousarray(
            np.broadcast_to(
                np.asarray(inputs["bv"], np.float32).reshape(1, DV), (P, DV)
            )
        ),
    }

    def prep_x(x, dt):
        # [s, d] f32 -> [ci, p, c, s_local] chunk-contiguous staging
        x = np.asarray(x, np.float32).reshape(NCH, CH, CD, P).transpose(0, 3, 2, 1)
        return x.astype(dt)

    in_maps = []
    for i in range(B):
        m = dict(weights)
        m["qT"] = prep_x(inputs["query"][i], kdt)
        m["kT"] = prep_x(inputs["key"][i], kdt)
        m["vT"] = prep_x(inputs["value"][i], bf)
        in_maps.append(m)
    return in_maps


def kernel(**inputs):
    from concourse.bass_utils import run_bass_kernel_spmd

    if "nc" not in _cache:
        _cache["nc"] = build(S)
    nc = _cache["nc"]
    in_maps = make_in_maps(inputs, S)
    res = run_bass_kernel_spmd(nc, in_maps, core_ids=list(range(B)))
    return np.stack([r["out"] for r in res.results], axis=0)


# revision 42
# speedup vs baseline: 1.0360x; 1.0303x over previous
"""Single-head attention (B=8, S=4096, D=1024, DK=DV=128) on 8 TRN2 NeuronCores.

Sharding: data-parallel over batch - one batch element per core, the three
Linear weights replicated. No collectives.

Measured (core 0 traced, 8 cores running): ~185-190us at the chip's
nominal clock (the chip's P0 power state adds up to ~20% run-to-run),
rel_err 1.10e-2 vs the f32 reference (tolerance 2e-2); v2 baseline was
212-221us at rel_err 1.7e-3.

Trace-driven changes from the v2 baseline:

1. ScalarE is the steady-state bottleneck: each [128,1024] exp ACTIVATE
   costs (N+352)/1.2 ns and the per-call overhead never pipelines away,
   so 128 ACTs = ~129us effective minimum. Everything else is scheduled
   around keeping that ACT stream dense:
   - all x-chunk load DMA triggers moved off the scalar queue onto the
     sync HWDGE ring only (one ring's DMA already fans out to all 16 SDMA
     engines; sequential c-splits keep projection subtile deps early);
     weights/biases on gpsimd SWDGE.
   - deferred-PV pipeline: each group's exp output (pt) is stashed in a
     deep SBUF pool (34 tiles) and its P^T@[V|1] matmuls are emitted LAG=3
     groups later (2*LAG at q-block boundaries, covering the attn_end DVE
     chain), so the PE-FIFO head never waits on a fresh ACT or on DVE and
     the ACT chain depends only on scores matmuls. Steady state measures
     ~1000ns/ACT (the floor).
   - qb0's scores+exp ride the DMA-bound load front SOLO, so attn_end(0)
     fires right at front-end and qb1's deferred PV (ops-WAR-gated on it)
     overlaps qb1's own gapless exp sweep instead of piling up behind a
     late attn_end (interleaving qb0+qb1 in the front was measured worse:
     it pushes attn_end(0) ~16us later and stalls ScalarE ~6us).

2. FP8_QK: query/key activations AND Wq/Wk are cast host-side to fp8-e4m3
   (TRN float8e4; values well under the 240 max). The Q/K projections run
   as fp8 DoubleRow matmuls (contraction pairs of d-chunks, 2 MACs/cycle):
   4 pair-MMs per 512-block instead of 8, halving Q/K projection PE time,
   and the Q/K input DMA bytes halve (the front is HBM-bound at ~300-410
   GB/s/core, so 16MB instead of 24MB directly shortens it). V path stays
   bf16: the output is a near-uniform softmax average, so V-side
   quantization error does not average out (numpy-sim: V-fp8 1.9e-2 vs
   QK-fp8 1.1e-2, both calibrated within 1% of HW).

3. Host staging is chunk-contiguous: x^T is pre-arranged per load-chunk as
   [ci][p][c*CH+s] so each chunk DMA is 128 descriptors x 4-16KB
   contiguous; measured DMA rate rose from ~300 to ~410 GB/s.

4. A ~90-matmul warm-up burst opens the PE HAM clock gate (1.2->2.4GHz)
   before the first real projections; later q-chunk projections are
   emitted one 512-block (4 DoubleRow pair-MMs) per sweep inside the
   attention stream where PE has slack.

Attention machinery unchanged from v2: transposed-score layout, exp on
ScalarE in [128,1024] calls, P^T @ [V | 1] PSUM accumulation with the ones
column producing the softmax denominator (ops), fused DVE normalize, with
sps 3-deep x [128,1024] (6 banks) + ops 2 banks = 8 PSUM banks.
"""

import math
import os

import numpy as np

B, S, D, DK, DV = 8, 4096, 1024, 128, 128
P = 128
SB = 512  # q-block width (attention) and projection block
CH = 1024  # load chunk (sequence cols per stage load)
CD = D // P  # 8 d-chunks
NSB = S // SB  # 8 q-blocks
NKC = S // P  # 32 key chunks
NCH = S // CH  # 4 load chunks per tensor
JPB = SB // P  # 4 q-subchunks per block
SCALE = 1.0 / math.sqrt(DK)

FP8_QK = os.environ.get("FP8_QK", "1") != "0"  # fp8-e4m3 x_q/x_k/Wq/Wk + DoubleRow projections

_cache = {}

# kept for test.py compat (unused)
XBAR_DUAL = False
SWDGE_QUEUES = 1


def _emit(tc, aps):
    from concourse import mybir

    nc = tc.nc
    bf16 = mybir.dt.bfloat16
    f32 = mybir.dt.float32

    qT, kT, vT, wq, wk, wv, bq, bk, bv, out = aps

    out_ap = out.rearrange("(nb j p) d -> nb p j d", p=P, j=JPB)

    from contextlib import ExitStack

    with ExitStack() as ctx:
        consts = ctx.enter_context(tc.tile_pool(name="consts", bufs=1))
        qkv = ctx.enter_context(tc.tile_pool(name="qkv", bufs=1))
        qtp = ctx.enter_context(tc.tile_pool(name="qt", bufs=NSB))
        stagep = ctx.enter_context(tc.tile_pool(name="stage", bufs=6))
        # deep pt pool: one q-block's full stash plus in-flight tiles
        ptp = ctx.enter_context(tc.tile_pool(name="pt", bufs=34))
        outp = ctx.enter_context(tc.tile_pool(name="outp", bufs=3))
        smallp = ctx.enter_context(tc.tile_pool(name="small", bufs=8))
        psump = ctx.enter_context(tc.tile_pool(name="ps", bufs=2, space="PSUM"))

        # --- constants ---
        if FP8_QK:
            f8 = mybir.dt.float8e4
            wq_sb = consts.tile([P, CD // 2, 2, DK], f8)
            wk_sb = consts.tile([P, CD // 2, 2, DK], f8)
        else:
            wq_sb = consts.tile([P, CD, DK], bf16)
            wk_sb = consts.tile([P, CD, DK], bf16)
        wv_sb = consts.tile([P, CD, DV], bf16)
        # wk first on sync (needed by the first projections), rest on gpsimd
        nc.sync.dma_start(out=wk_sb, in_=wk)
        nc.gpsimd.dma_start(out=wv_sb, in_=wv)
        nc.gpsimd.dma_start(out=wq_sb, in_=wq)
        bq_sb = consts.tile([P, 1], f32)
        bk_sb = consts.tile([P, 1], f32)
        bv_sb = consts.tile([P, DV], f32)
        nc.gpsimd.dma_start(out=bq_sb, in_=bq)
        nc.gpsimd.dma_start(out=bk_sb, in_=bk)
        nc.gpsimd.dma_start(out=bv_sb, in_=bv)

        # warm the exp table set while loads stream
        warm_in = consts.tile([P, 8], f32)
        warm_out = consts.tile([P, 8], f32)
        nc.vector.memset(warm_in, 0.0)
        nc.scalar.activation(warm_out, warm_in, mybir.ActivationFunctionType.Exp)

        # persistent per-core tensors
        kt_sb = qkv.tile([P, S], bf16)  # K^T  [dk, s]
        vp_sb = qkv.tile([P, NKC, DV + 1], bf16)  # V' natural [s%128, chunk, dv+1]
        nc.vector.memset(vp_sb[:, :, DV : DV + 1], 1.0)
        qts = [qtp.tile([P, SB], bf16, tag="qt", name=f"qt{i}") for i in range(NSB)]

        def load_chunk(src_ap, ci, nm, dt, quarters=False, eng=None):
            # c-splits let the projections' subtile deps fire as each split
            # lands.  eng picks the HWDGE ring: per-DMA fixed overheads
            # serialize per ring, so the front splits across sync AND scalar
            # (ScalarE's FIFO is ACT-free until the first exp).
            st = stagep.tile([P, CD, CH], dt, tag="stage", name=f"st_{nm}{ci}")
            sl = src_ap[ci]
            step = CD // 4 if quarters else CD // 2
            for c0 in range(0, CD, step):
                e = eng if eng is not None else nc.sync
                e.dma_start(out=st[:, c0 : c0 + step, :], in_=sl[:, c0 : c0 + step, :])
            return st

        def load_chunk_dual(src_ap, ci, nm, dt):
            # halves on both rings in parallel
            st = stagep.tile([P, CD, CH], dt, tag="stage", name=f"st_{nm}{ci}")
            sl = src_ap[ci]
            h = CD // 2
            nc.sync.dma_start(out=st[:, 0:h, :], in_=sl[:, 0:h, :])
            nc.scalar.dma_start(out=st[:, h:CD, :], in_=sl[:, h:CD, :])
            return st

        def project_x(st, ci, w_sb, b_sb, dst, h_lo, h_hi, nm):
            # dst: callable sb -> (out_ap) receiving the biased block
            for h in range(h_lo, h_hi):
                sb = ci * (CH // SB) + h
                ps = psump.tile([P, SB], f32, tag="sps", bufs=3, name=f"{nm}ps{sb}")
                if FP8_QK:
                    for c2 in range(CD // 2):
                        nc.tensor.matmul(
                            ps,
                            w_sb[:, c2, :, :],
                            st[:, 2 * c2 : 2 * c2 + 2, h * SB : (h + 1) * SB],
                            start=(c2 == 0),
                            stop=(c2 == CD // 2 - 1),
                            perf_mode=mybir.MatmulPerfMode.DoubleRow,
                        )
                else:
                    for c in range(CD):
                        nc.tensor.matmul(
                            ps,
                            w_sb[:, c, :],
                            st[:, c, h * SB : (h + 1) * SB],
                            start=(c == 0),
                            stop=(c == CD - 1),
                        )
                nc.vector.tensor_scalar_add(dst(sb), ps, b_sb)

        def project_kt(st, ci, h_lo=0, h_hi=CH // SB):
            project_x(
                st, ci, wk_sb, bk_sb,
                lambda sb: kt_sb[:, sb * SB : (sb + 1) * SB], h_lo, h_hi, "k",
            )

        def project_qt(st, ci, h_lo=0, h_hi=CH // SB):
            project_x(st, ci, wq_sb, bq_sb, lambda sb: qts[sb], h_lo, h_hi, "q")

        def project_v(st, ci, j_lo=0, j_hi=CH // P):
            # 4 j-outputs packed per [128, 512] psum tile; start=True only on
            # the first matmul per bank (whole-bank has_written clear), later
            # j's first write overwrites on cleared bits.
            for j4 in range(j_lo, j_hi, 4):
                vps = psump.tile(
                    [P, SB], f32, tag="sps", bufs=3, name=f"vps{ci}_{j4}"
                )
                for j in range(j4, j4 + 4):
                    jj = j - j4
                    for c in range(CD):
                        nc.tensor.matmul(
                            vps[:, jj * DV : (jj + 1) * DV],
                            st[:, c, j * P : (j + 1) * P],
                            wv_sb[:, c, :],
                            start=(jj == 0 and c == 0),
                            stop=(c == CD - 1),
                        )
                kk0 = ci * (CH // P) + j4
                nc.vector.tensor_copy(
                    vp_sb[:, kk0 : kk0 + 4, 0:DV],
                    vps.rearrange("p (j d) -> p j d", j=4),
                )

        # --- attention emission helpers ---
        # key chunks grouped 2 per exp call ([128, 1024] ACTIVATEs); the
        # smaller group buys a 3-deep sps rotation (3x2=6 banks) that
        # decouples the scores matmuls from exp WAR jitter.
        #
        # Deferred-PV pipeline: attn_scores stashes each group's exp'd
        # probabilities (pt) in a deep SBUF pool; attn_pv consumes them
        # later - each q-block's PV matmuls ride the NEXT q-block's
        # scores/exp stream.  The ScalarE ACT chain then depends only on
        # scores matmuls (never on PV/ops/DVE), and two q-blocks' worth of
        # exp fits inside the DMA-bound load front with only 2 ops banks.
        groups = [(g * 2, 2) for g in range(NKC // 2)]
        ngrp = len(groups)
        qb_ops = {}
        pt_stash = {}

        def attn_begin(qb):
            opsA = psump.tile([P, 2, DV + 1], f32, tag="ops", bufs=2, name=f"opsA{qb}")
            opsB = psump.tile([P, 2, DV + 1], f32, tag="ops", bufs=2, name=f"opsB{qb}")
            qb_ops[qb] = [opsA[:, 0, :], opsA[:, 1, :], opsB[:, 0, :], opsB[:, 1, :]]

        def attn_scores(qb, g_lo, g_hi):
            for gi in range(g_lo, g_hi):
                k0, gn = groups[gi]
                sps = psump.tile(
                    [P, gn * SB], f32, tag="sps", bufs=3, name=f"sps{qb}_{gi}"
                )
                for h in range(gn):
                    kk = k0 + h
                    nc.tensor.matmul(
                        sps[:, h * SB : (h + 1) * SB],
                        kt_sb[:, kk * P : (kk + 1) * P],
                        qts[qb],
                        start=True,
                        stop=True,
                    )
                pt = ptp.tile([P, gn * SB], bf16, tag="pt", name=f"pt{qb}_{gi}")
                nc.scalar.activation(
                    pt, sps, mybir.ActivationFunctionType.Exp, scale=SCALE
                )
                pt_stash[(qb, gi)] = pt

        def attn_pv(qb, g_lo, g_hi):
            ops = qb_ops[qb]
            for gi in range(g_lo, g_hi):
                k0, gn = groups[gi]
                pt = pt_stash.pop((qb, gi))
                for h in range(gn):
                    kk = k0 + h
                    for j in range(JPB):
                        # start=True clears has_written for the WHOLE bank, so
                        # only the first matmul per bank (j=0 / j=2) may set it;
                        # the partner tile's first write lands on cleared bits
                        # and overwrites (per-element has_written semantics).
                        nc.tensor.matmul(
                            ops[j],
                            pt[:, h * SB + j * P : h * SB + (j + 1) * P],
                            vp_sb[:, kk, :],
                            start=(gi == 0 and h == 0 and j % 2 == 0),
                            stop=(gi == ngrp - 1 and h == gn - 1),
                        )

        def attn_end(qb):
            ops = qb_ops.pop(qb)
            ostage = outp.tile([P, JPB, DV], f32, tag="ostage", name=f"ostage{qb}")
            for j in range(JPB):
                recip = smallp.tile([P, 1], f32, tag="recip", name=f"recip{qb}_{j}")
                nc.vector.reciprocal(recip, ops[j][:, DV : DV + 1])
                nc.vector.scalar_tensor_tensor(
                    ostage[:, j, :],
                    ops[j][:, 0:DV],
                    recip,
                    bv_sb,
                    mybir.AluOpType.mult,
                    mybir.AluOpType.add,
                )
                if j % 2 == 1:  # store halves as they complete (shorter tail);
                    nc.sync.dma_start(
                        out=out_ap[qb][:, j - 1 : j + 1, :],
                        in_=ostage[:, j - 1 : j + 1, :],
                    )

        # Global scores / deferred-PV streams.  The PV stream trails the
        # scores stream by LAG groups across the WHOLE kernel, so (a) PV MMs
        # at the PE-FIFO head are never waiting on a fresh exp or on the
        # attn_end DVE chain, (b) there is no PV drain lump at q-block
        # boundaries or at kernel end (PV finishes LAG groups after the last
        # ACT), and (c) attn_end/attn_begin ride mid-stream.
        LAG = 3
        sc_pos = {"n": 0}
        pv_pos = {"n": 0}
        sc_prog = {}  # qb -> groups of scores emitted

        def pv_advance_one(drain=False):
            qb, g = divmod(pv_pos["n"], ngrp)
            if qb >= NSB or (qb, g) not in pt_stash:
                return False  # stream exhausted / that exp not emitted yet
            need = g + 1 + (LAG if g > 0 else 2 * LAG)
            if not drain and sc_prog.get(qb, 0) < min(need, ngrp):
                # keep LAG groups between a qb's ACTs and its PV; double at
                # the q-block boundary so the previous block's attn_end DVE
                # chain finishes before this block's first (ops-WAR-gated) PV
                return False
            attn_pv(qb, g, g + 1)
            pv_pos["n"] += 1
            if g + 1 == ngrp:
                attn_end(qb)
                if qb + 1 < NSB:
                    attn_begin(qb + 1)
            return True

        def sc(qb, g):
            attn_scores(qb, g, g + 1)
            sc_pos["n"] += 1
            sc_prog[qb] = g + 1
            while pv_pos["n"] < sc_pos["n"] - LAG:
                if not pv_advance_one():
                    break

        # --- software pipeline in emission order (engines run their streams
        # FIFO, so emission order IS the per-engine execution order) ---
        kdt = mybir.dt.float8e4 if FP8_QK else bf16
        stk0 = load_chunk(kT, 0, "k", kdt, quarters=True)
        stq0 = load_chunk(qT, 0, "q", kdt, quarters=True)
        stv0 = load_chunk(vT, 0, "v", kdt, quarters=True)

        # PE warm-up burst: dependency-free matmuls open the HAM clock-gate
        # (1.2 -> 2.4 GHz) before the first real projections.
        warm_w = consts.tile([P, 64], bf16)
        nc.vector.memset(warm_w, 0.0)
        wps = psump.tile([P, SB], f32, tag="sps", bufs=3, name="warmps")
        for _ in range(90):
            nc.tensor.matmul(wps[0:64, 0:64], warm_w, warm_w, start=True, stop=True)

        # first chunk's projections at block granularity so the first
        # attention groups fire as soon as kc 0..3 are projected
        project_kt(stk0, 0, 0, 1)  # kc 0..3
        project_qt(stq0, 0)  # qt[0], qt[1]
        project_v(stv0, 0, 0, 4)  # vp 0..3

        # Load front: qb0's, qb1's and (trailing 6 groups) qb2's scores+exp
        # all ride the DMA-bound window - 48 of the 128 ACTs complete before
        # the steady state begins, soaking up the chunk-wait gaps.
        attn_begin(0)
        for g in range(0, 2):  # kc 0..3
            sc(0, g)
        project_kt(stk0, 0, 1, 2)  # kc 4..7
        project_v(stv0, 0, 4, 8)  # vp 4..7
        stk = load_chunk(kT, 1, "k", kdt)
        stv = load_chunk(vT, 1, "v", kdt)
        for g in range(2, 4):  # kc 4..7
            sc(0, g)
        project_kt(stk, 1)
        project_v(stv, 1)
        stq1 = load_chunk(qT, 1, "q", kdt)
        stk = load_chunk(kT, 2, "k", kdt)
        stv = load_chunk(vT, 2, "v", kdt)
        for g in range(4, 6):  # kc 8..11, needs chunk 1
            sc(0, g)
        project_qt(stq1, 1)  # qts[2], qts[3]
        for g in range(6, 8):
            sc(0, g)
        project_kt(stk, 2)
        project_v(stv, 2)
        stk = load_chunk(kT, 3, "k", kdt)
        stv = load_chunk(vT, 3, "v", kdt)
        for g in range(8, 12):  # kc 16..23, needs chunk 2
            sc(0, g)
        project_kt(stk, 3)
        project_v(stv, 3)
        stq2 = load_chunk(qT, 2, "q", kdt)
        for g in range(12, ngrp):  # kc 24..31
            sc(0, g)
        stq3 = load_chunk(qT, 3, "q", kdt)  # before out-stores clog the ring

        # steady state: one continuous scores sweep per remaining q-block;
        # the deferred-PV stream self-advances LAG groups behind; one
        # qt-projection 512-block (4 DoubleRow pair-MMs, ~1us PE) inserted
        # per sweep.
        for qs in range(1, NSB):
            for g in range(ngrp):
                sc(qs, g)
                if g == 3:
                    if qs == 3:
                        project_qt(stq2, 2, 0, 1)  # qts[4]
                    elif qs == 4:
                        project_qt(stq2, 2, 1, 2)  # qts[5]
                    elif qs == 5:
                        project_qt(stq3, 3, 0, 1)  # qts[6]
                    elif qs == 6:
                        project_qt(stq3, 3, 1, 2)  # qts[7]
        while pv_advance_one(drain=True):
            pass


def build(s_len=S):
    import concourse.tile as tile
    from concourse import bacc, mybir

    nc = bacc.Bacc(
        "TRN2",
        target_bir_lowering=False,
        debug=False,
        enable_asserts=False,
        num_devices=8,
    )
    f32 = mybir.dt.float32
    bf16 = mybir.dt.bfloat16
    kdt = mybir.dt.float8e4 if FP8_QK else bf16
    if FP8_QK:
        wshape = [P, CD // 2, 2, DK]
        wdt = mybir.dt.float8e4
    else:
        wshape = [P, CD, DK]
        wdt = bf16
    aps = [
        nc.dram_tensor("qT", [NCH, P, CD, CH], kdt, kind="ExternalInput").ap(),
        nc.dram_tensor("kT", [NCH, P, CD, CH], kdt, kind="ExternalInput").ap(),
        nc.dram_tensor("vT", [NCH, P, CD, CH], kdt, kind="ExternalInput").ap(),
        nc.dram_tensor("Wq", wshape, wdt, kind="ExternalInput").ap(),
        nc.dram_tensor("Wk", wshape, wdt, kind="ExternalInput").ap(),
        nc.dram_tensor("Wv", [P, CD, DV], bf16, kind="ExternalInput").ap(),
        nc.dram_tensor("bq", [DK, 1], f32, kind="ExternalInput").ap(),
        nc.dram_tensor("bk", [DK, 1], f32, kind="ExternalInput").ap(),
        nc.dram_tensor("bv", [P, DV], f32, kind="ExternalInput").ap(),
        nc.dram_tensor("out", [S, DV], f32, kind="ExternalOutput").ap(),
    ]
    with tile.TileContext(nc) as tc:
        _emit(tc, aps)
    nc.compile()
    return nc


def make_in_maps(inputs, s_len=S):
    import ml_dtypes

    bf = ml_dtypes.bfloat16
    f8 = ml_dtypes.float8_e4m3
    kdt = f8 if FP8_QK else bf

    def prep_w(w):
        # [d, k] -> [p, c, k] with d = c*128 + p
        w = np.asarray(w, np.float32).reshape(CD, P, -1).transpose(1, 0, 2)
        return np.ascontiguousarray(w).astype(bf)

    def prep_w_pair(w):
        # [d, k] -> [p, c2, i, k] with d = (2*c2 + i)*128 + p
        w = np.asarray(w, np.float32).reshape(CD // 2, 2, P, -1).transpose(2, 0, 1, 3)
        return np.ascontiguousarray(w).astype(f8)

    prep_wqk = prep_w_pair if FP8_QK else prep_w

    weights = {
        "Wq": prep_wqk(inputs["Wq"]),
        "Wk": prep_wqk(inputs["Wk"]),
        "Wv": prep_w(inputs["Wv"]),
        "bq": np.ascontiguousarray(inputs["bq"], dtype=np.float32).reshape(DK, 1),
        "bk": np.ascontiguousarray(inputs["bk"], dtype=np.float32).reshape(DK, 1),
        "bv": np.ascontiguousarray(
            np.broadcast_to(
                np.asarray(inputs["bv"], np.float32).reshape(1, DV), (P, DV)
            )
        ),
    }

    def prep_x(x, dt):
        # [s, d] f32 -> [ci, p, c, s_local] chunk-contiguous staging
        x = np.asarray(x, np.float32).reshape(NCH, CH, CD, P).transpose(0, 3, 2, 1)
        return x.astype(dt)

    in_maps = []
    for i in range(B):
        m = dict(weights)
        m["qT"] = prep_x(inputs["query"][i], kdt)
        m["kT"] = prep_x(inputs["key"][i], kdt)
        m["vT"] = prep_x(inputs["value"][i], kdt)
        in_maps.append(m)
    return in_maps


def kernel(**inputs):
    from concourse.bass_utils import run_bass_kernel_spmd

    if "nc" not in _cache:
        _cache["nc"] = build(S)
    nc = _cache["nc"]
    in_maps = make_in_maps(inputs, S)
    res = run_bass_kernel_spmd(nc, in_maps, core_ids=list(range(B)))
    return np.stack([r["out"] for r in res.results], axis=0)
